# revision 6
# baseline (speedup 1.0000x reference)
"""Trainium2 Bass kernel for nn_DiffNet (2-layer LSTM encoder/decoder + FC head).

Sharding: tensor-parallel over the hidden/gate dimension across 8 NeuronCores.
Core k owns hidden rows [k*128, (k+1)*128) of both LSTM layers (and the
matching rows of each of the 4 gates) plus the matching rows of fc_w1 and
columns of fc_w2.  Activations are stored as [hidden_partitions, batch] so the
full batch (256) is the matmul moving dimension; float32r matmuls then run at
1 cycle/row.  Hidden states are exchanged once per layer per step with an
8-rank AllGather; the decode FC contribution is computed K-sharded and its
partial products ride a third AllGather, summed on every core so `est` is
replicated.

Self-contained: hardcodes all shapes; host-side numpy only reshapes/slices.
"""

import os

import numpy as np

L = 2
H = 1024
XD = 192
YD = 64
IN = XD + YD  # 256
B = 256
PRE_LEN = int(os.environ.get("DIFFNET_PRE", "64"))
FWD_LEN = int(os.environ.get("DIFFNET_FWD", "48"))
NCORES = 8
SL = H // NCORES  # 128 hidden rows per core
KT_H = H // 128  # 8 K-tiles to contract over a full hidden vector
NGATE = 4

_CACHE = {}


def _shard_host(inputs):
    """Build per-core input dicts (numpy only: slice / transpose / reshape)."""
    f32 = np.float32

    pre_x = np.asarray(inputs["pre_x"], f32)
    pre_y = np.asarray(inputs["pre_y"], f32)
    fwd_x = np.asarray(inputs["forward_x"], f32)

    # Encoder input, step-major, [t, p(128), kt(2), b] so the DMA is contiguous
    xy = np.concatenate([pre_x, pre_y], axis=2)  # (PRE, B, IN)
    xpre = (
        xy.transpose(0, 2, 1)  # (PRE, IN, B)
        .reshape(PRE_LEN, 2, 128, B)
        .transpose(0, 2, 1, 3)  # (PRE, 128, 2, B)
        .copy()
    )
    # Decoder exogenous input: [t, in(192), b]
    xfwd = fwd_x.transpose(0, 2, 1).copy()  # (FWD, 192, B)

    w_ih_0 = np.asarray(inputs["w_ih_0"], f32).reshape(NGATE, H, IN)
    w_hh_0 = np.asarray(inputs["w_hh_0"], f32).reshape(NGATE, H, H)
    w_ih_1 = np.asarray(inputs["w_ih_1"], f32).reshape(NGATE, H, H)
    w_hh_1 = np.asarray(inputs["w_hh_1"], f32).reshape(NGATE, H, H)
    b0 = (np.asarray(inputs["b_ih_0"], f32) + np.asarray(inputs["b_hh_0"], f32)).reshape(NGATE, H)
    b1 = (np.asarray(inputs["b_ih_1"], f32) + np.asarray(inputs["b_hh_1"], f32)).reshape(NGATE, H)
    fc_w1 = np.asarray(inputs["fc_w1"], f32)
    fc_b1 = np.asarray(inputs["fc_b1"], f32)
    fc_w2 = np.asarray(inputs["fc_w2"], f32)
    fc_b2 = np.asarray(inputs["fc_b2"], f32)

    def lhsT_hid(w, k):
        """(4, H, K) gate-major weight -> lhsT [128, KT, 4, 128] for core k."""
        sl = w[:, k * SL : (k + 1) * SL, :]  # (4, 128, K)
        kdim = sl.shape[2]
        kt = kdim // 128
        return (
            sl.transpose(2, 0, 1)  # (K, 4, 128)
            .reshape(kt, 128, NGATE, SL)
            .transpose(1, 0, 2, 3)  # (128, kt, 4, 128)
            .reshape(128, kt * NGATE * SL)
            .copy()
        )

    maps = []
    for k in range(NCORES):
        sl = slice(k * SL, (k + 1) * SL)
        w0xT = lhsT_hid(w_ih_0, k)  # (128, 2*4*128)
        west = w_ih_0[:, sl, XD:]  # (4, 128, 64)
        westT = west.transpose(2, 0, 1).reshape(YD, NGATE * SL).copy()  # (64, 512)
        whh0T = lhsT_hid(w_hh_0, k)  # (128, 8*4*128)
        wih1T = lhsT_hid(w_ih_1, k)
        whh1T = lhsT_hid(w_hh_1, k)
        fw1 = fc_w1[sl, :]  # (128, 1024)
        fcw1T = (
            fw1.T.reshape(KT_H, 128, SL).transpose(1, 0, 2).reshape(128, KT_H * SL).copy()
        )
        fcw2T = fc_w2[:, sl].T.copy()  # (128, 64)
        m = {
            "xpre": xpre,
            "xfwd": xfwd,
            "w0xT": w0xT,
            "westT": westT,
            "whh0T": whh0T,
            "wih1T": wih1T,
            "whh1T": whh1T,
            "fcw1T": fcw1T,
            "fcw2T": fcw2T,
            "b0": b0[:, sl].T.copy(),  # (128, 4)
            "b1": b1[:, sl].T.copy(),
            "fcb1": fc_b1[sl].reshape(SL, 1).copy(),
            "fcb2": fc_b2.reshape(YD, 1).copy(),
            "lastyT": pre_y[-1].T.copy(),  # (64, 256)
        }
        maps.append(m)
    return maps


def _build_program():
    import concourse.bass as bass
    import concourse.mybir as mybir
    import concourse.tile as tile
    from concourse import bacc

    dt = mybir.dt
    AF = mybir.ActivationFunctionType
    F32 = dt.float32
    FR = dt.float32r  # rounded fp32: matmuls run at 1 cycle/row for N>=256

    nc = bacc.Bacc("TRN2", target_bir_lowering=False, debug=False, num_devices=NCORES)

    # ---- external I/O (everything feeding a matmul is declared float32r) ----
    t_xpre = nc.dram_tensor("xpre", [PRE_LEN, 128, 2, B], FR, kind="ExternalInput")
    t_xfwd = nc.dram_tensor("xfwd", [FWD_LEN, XD, B], FR, kind="ExternalInput")
    t_w0xT = nc.dram_tensor("w0xT", [128, 2 * NGATE * SL], FR, kind="ExternalInput")
    t_westT = nc.dram_tensor("westT", [YD, NGATE * SL], FR, kind="ExternalInput")
    t_whh0T = nc.dram_tensor("whh0T", [128, KT_H * NGATE * SL], FR, kind="ExternalInput")
    t_wih1T = nc.dram_tensor("wih1T", [128, KT_H * NGATE * SL], FR, kind="ExternalInput")
    t_whh1T = nc.dram_tensor("whh1T", [128, KT_H * NGATE * SL], FR, kind="ExternalInput")
    t_fcw1T = nc.dram_tensor("fcw1T", [128, KT_H * SL], FR, kind="ExternalInput")
    t_fcw2T = nc.dram_tensor("fcw2T", [128, YD], FR, kind="ExternalInput")
    t_b0 = nc.dram_tensor("b0", [128, NGATE], F32, kind="ExternalInput")
    t_b1 = nc.dram_tensor("b1", [128, NGATE], F32, kind="ExternalInput")
    t_fcb1 = nc.dram_tensor("fcb1", [SL, 1], F32, kind="ExternalInput")
    t_fcb2 = nc.dram_tensor("fcb2", [YD, 1], F32, kind="ExternalInput")
    t_lastyT = nc.dram_tensor("lastyT", [YD, B], F32, kind="ExternalInput")
    t_out = nc.dram_tensor("est_out", [FWD_LEN, YD, B], F32, kind="ExternalOutput")

    RG = [list(range(NCORES))]

    with tile.TileContext(nc) as tc:
        with (
            tc.tile_pool(name="const", bufs=1) as const,
            tc.tile_pool(name="xload", bufs=3) as xload,
            tc.tile_pool(name="state", bufs=2) as state,
            tc.tile_pool(name="gact", bufs=3) as gact,
            tc.tile_pool(name="hfull", bufs=2) as hfull,
            tc.tile_pool(name="psum", bufs=8, space="PSUM") as psum,
            tc.tile_pool(name="dbounce", bufs=4, space="DRAM") as dbounce,
            tc.tile_pool(name="dshared", bufs=4, space="DRAM") as dshared,
        ):
            # ---- load constants ----
            w0xT = const.tile([128, 2, NGATE, SL], FR)
            nc.sync.dma_start(out=w0xT, in_=t_w0xT.ap().rearrange("p (k g m) -> p k g m", k=2, g=NGATE))
            westT = const.tile([YD, NGATE, SL], FR)
            nc.sync.dma_start(out=westT, in_=t_westT.ap().rearrange("p (g m) -> p g m", g=NGATE))
            whh0T = const.tile([128, KT_H, NGATE, SL], FR)
            nc.sync.dma_start(out=whh0T, in_=t_whh0T.ap().rearrange("p (k g m) -> p k g m", k=KT_H, g=NGATE))
            wih1T = const.tile([128, KT_H, NGATE, SL], FR)
            nc.sync.dma_start(out=wih1T, in_=t_wih1T.ap().rearrange("p (k g m) -> p k g m", k=KT_H, g=NGATE))
            whh1T = const.tile([128, KT_H, NGATE, SL], FR)
            nc.sync.dma_start(out=whh1T, in_=t_whh1T.ap().rearrange("p (k g m) -> p k g m", k=KT_H, g=NGATE))
            fcw1T = const.tile([128, KT_H, SL], FR)
            nc.sync.dma_start(out=fcw1T, in_=t_fcw1T.ap().rearrange("p (k m) -> p k m", k=KT_H))
            fcw2T = const.tile([128, YD], FR)
            nc.sync.dma_start(out=fcw2T, in_=t_fcw2T.ap())
            b0 = const.tile([128, NGATE], F32)
            nc.sync.dma_start(out=b0, in_=t_b0.ap())
            b1 = const.tile([128, NGATE], F32)
            nc.sync.dma_start(out=b1, in_=t_b1.ap())
            fcb1 = const.tile([SL, 1], F32)
            nc.sync.dma_start(out=fcb1, in_=t_fcb1.ap())
            fcb2 = const.tile([YD, 1], F32)
            nc.sync.dma_start(out=fcb2, in_=t_fcb2.ap())

            # ---- persistent state ----
            est = const.tile([YD, B], F32)  # replicated running estimate
            nc.sync.dma_start(out=est, in_=t_lastyT.ap())
            c0 = const.tile([128, B], F32)
            nc.vector.memset(c0, 0.0)
            c1 = const.tile([128, B], F32)
            nc.vector.memset(c1, 0.0)
            h0f = None  # gathered h0 of previous step [128, KT_H, B] (f32r)
            h1f = None  # gathered h1 of previous step

            def allgather(slice_ap, nparts, dtype):
                inb = dbounce.tile([nparts, B], dtype, tag="agin", name="agin")
                nc.sync.dma_start(out=inb, in_=slice_ap)
                outb = dshared.tile([NCORES * nparts, B], dtype, tag="agout",
                                    name="agout", addr_space="Shared")
                nc.gpsimd.collective_compute(
                    "AllGather",
                    mybir.AluOpType.bypass,
                    replica_groups=RG,
                    ins=[inb[:].opt()],
                    outs=[outb[:].opt()],
                )
                return outb

            def gather_to_sbuf(outb, nparts, dtype):
                """DMA gathered DRAM [NCORES*nparts, B] into SBUF [nparts, NCORES, B]
                k-tile by k-tile so consumers can start early."""
                full = hfull.tile([nparts, NCORES, B], dtype, tag=f"full{nparts}",
                                  name="hfull")
                src = outb[:].rearrange("(k p) b -> p k b", p=nparts)
                for k in range(NCORES):
                    nc.sync.dma_start(out=full[:, k, :], in_=src[:, k, :])
                return full

            def lstm_halfstep(zp, bias, cprev, tagp):
                """Gate activations + cell update. zp: 4 PSUM tiles [128,B].
                Returns (c_new, h_new); h_new is written as float32r."""
                gi = gact.tile([128, B], F32, tag="gi", name="gi")
                gf = gact.tile([128, B], F32, tag="gf", name="gf")
                gg = gact.tile([128, B], F32, tag="gg", name="gg")
                go = gact.tile([128, B], F32, tag="go", name="go")
                nc.scalar.activation(gi, zp[0], AF.Sigmoid, bias=bias[:, 0:1])
                nc.scalar.activation(gf, zp[1], AF.Sigmoid, bias=bias[:, 1:2])
                nc.scalar.activation(gg, zp[2], AF.Tanh, bias=bias[:, 2:3])
                nc.scalar.activation(go, zp[3], AF.Sigmoid, bias=bias[:, 3:4])
                fc_ = gact.tile([128, B], F32, tag="fc_", name="fc_")
                nc.vector.tensor_mul(fc_, gf, cprev)
                ig = gact.tile([128, B], F32, tag="ig", name="ig")
                nc.vector.tensor_mul(ig, gi, gg)
                cnew = state.tile([128, B], F32, tag=tagp, name="cnew")
                nc.vector.tensor_add(cnew, fc_, ig)
                tc_ = gact.tile([128, B], F32, tag="tc_", name="tc_")
                nc.scalar.activation(tc_, cnew, AF.Tanh)
                hnew = state.tile([128, B], FR, tag=tagp + "h", name="hnew")
                nc.vector.tensor_mul(hnew, go, tc_)
                return cnew, hnew

            nsteps = PRE_LEN + FWD_LEN
            for s in range(nsteps):
                dec = s >= PRE_LEN
                # ---------- layer0 gate matmuls ----------
                z0 = [psum.tile([128, B], F32, tag="z", name=f"z0g{g}") for g in range(NGATE)]
                xt = xload.tile([128, 2, B], FR, tag="x", name="xt")
                if not dec:
                    nc.sync.dma_start(out=xt, in_=t_xpre.ap()[s])
                    for g in range(NGATE):
                        nc.tensor.matmul(z0[g], w0xT[:, 0, g, :], xt[:, 0, :],
                                         start=True, stop=False)
                        nc.tensor.matmul(z0[g], w0xT[:, 1, g, :], xt[:, 1, :],
                                         start=False, stop=(s == 0))
                        if s > 0:
                            for k in range(KT_H):
                                nc.tensor.matmul(
                                    z0[g], whh0T[:, k, g, :], h0f[:, k, :],
                                    start=False, stop=(k == KT_H - 1),
                                )
                else:
                    sf = s - PRE_LEN
                    nc.sync.dma_start(out=xt[:, 0, :], in_=t_xfwd.ap()[sf, 0:128, :])
                    nc.sync.dma_start(out=xt[0:64, 1, :], in_=t_xfwd.ap()[sf, 128:XD, :])
                    # FC head: est_s = fc(h1_{s-1}) + est_{s-1}; h1f holds the
                    # gathered previous h1.
                    up = psum.tile([SL, B], F32, tag="z", name="up")
                    for k in range(KT_H):
                        nc.tensor.matmul(up, fcw1T[:, k, :], h1f[:, k, :],
                                         start=(k == 0), stop=(k == KT_H - 1))
                    u = gact.tile([SL, B], FR, tag="u", name="u")
                    nc.scalar.activation(u, up, AF.Tanh, bias=fcb1[:, 0:1])
                    pp = psum.tile([YD, B], F32, tag="z", name="pp")
                    nc.tensor.matmul(pp, fcw2T[:, :], u, start=True, stop=True)
                    psb = gact.tile([YD, B], F32, tag="psb", name="psb")
                    nc.vector.tensor_copy(psb, pp)
                    pg = allgather(psb[:], YD, F32)
                    parts = gather_to_sbuf(pg, YD, F32)  # [64, 8, 256]
                    s1 = gact.tile([YD, 4, B], F32, tag="s1", name="s1")
                    nc.vector.tensor_add(s1, parts[:, 0:4, :], parts[:, 4:8, :])
                    s2 = gact.tile([YD, 2, B], F32, tag="s2", name="s2")
                    nc.vector.tensor_add(s2, s1[:, 0:2, :], s1[:, 2:4, :])
                    s3 = gact.tile([YD, B], F32, tag="s3", name="s3")
                    nc.vector.tensor_add(s3, s2[:, 0, :], s2[:, 1, :])
                    estn = state.tile([YD, B], F32, tag="est", name="estn")
                    nc.vector.tensor_add(estn, est, s3)
                    nc.vector.tensor_scalar_add(estn, estn, fcb2[:, 0:1])
                    est = estn
                    est_r = state.tile([YD, B], FR, tag="estr", name="est_r")
                    nc.vector.tensor_copy(est_r, est)
                    nc.sync.dma_start(out=t_out.ap()[sf], in_=est)
                    for g in range(NGATE):
                        nc.tensor.matmul(z0[g], w0xT[:, 0, g, :], xt[:, 0, :],
                                         start=True, stop=False)
                        nc.tensor.matmul(z0[g], w0xT[0:64, 1, g, :], xt[0:64, 1, :],
                                         start=False, stop=False)
                        for k in range(KT_H):
                            nc.tensor.matmul(z0[g], whh0T[:, k, g, :], h0f[:, k, :],
                                             start=False, stop=False)
                        nc.tensor.matmul(z0[g], westT[:, g, :], est_r,
                                         start=False, stop=True)

                c0, h0k = lstm_halfstep(z0, b0, c0, "c0")
                g0 = allgather(h0k[:], 128, FR)
                h0f = gather_to_sbuf(g0, 128, FR)

                # ---------- layer1 ----------
                z1 = [psum.tile([128, B], F32, tag="z", name=f"z1g{g}") for g in range(NGATE)]
                for g in range(NGATE):
                    if s > 0:
                        for k in range(KT_H):
                            nc.tensor.matmul(z1[g], whh1T[:, k, g, :], h1f[:, k, :],
                                             start=(k == 0), stop=False)
                    for k in range(KT_H):
                        nc.tensor.matmul(z1[g], wih1T[:, k, g, :], h0f[:, k, :],
                                         start=(s == 0 and k == 0), stop=(k == KT_H - 1))
                c1, h1k = lstm_halfstep(z1, b1, c1, "c1")
                if s < nsteps - 1:
                    g1 = allgather(h1k[:], 128, FR)
                    h1f = gather_to_sbuf(g1, 128, FR)

    nc.compile()
    return nc


def kernel(**inputs) -> np.ndarray:
    from concourse.bass_utils import run_bass_kernel_spmd

    key = "prog"
    if key not in _CACHE:
        _CACHE[key] = _build_program()
    nc = _CACHE[key]

    in_maps = _shard_host(inputs)
    res = run_bass_kernel_spmd(nc, in_maps, core_ids=list(range(NCORES)))
    est = np.asarray(res.results[0]["est_out"])  # (FWD, YD, B)
    return est.transpose(0, 2, 1).astype(np.float32).copy()  # (FWD, B, YD)


# revision 12
# speedup vs baseline: 187.0634x; 187.0634x over previous
"""Trainium2 Bass kernel for nn_DiffNet (2-layer LSTM encoder/decoder + FC head).

Sharding: tensor-parallel over the hidden/gate dimension across 8 NeuronCores.
Core k owns hidden rows [k*128, (k+1)*128) of both LSTM layers (and the
matching rows of each of the 4 gates) plus the matching rows of fc_w1 and
columns of fc_w2.  Activations are stored as [hidden_partitions, batch] so the
full batch (256) is the matmul moving dimension; float32r matmuls then run at
1 cycle/row.  Hidden states are exchanged once per layer per step with an
8-rank AllGather; the decode FC contribution is computed K-sharded and its
partial products ride a third AllGather, summed on every core so `est` is
replicated.

Self-contained: hardcodes all shapes; host-side numpy only reshapes/slices.
"""

import os

import numpy as np

L = 2
H = 1024
XD = 192
YD = 64
IN = XD + YD  # 256
B = 256
PRE_LEN = int(os.environ.get("DIFFNET_PRE", "64"))
FWD_LEN = int(os.environ.get("DIFFNET_FWD", "48"))
NCORES = 8
SL = H // NCORES  # 128 hidden rows per core
KT_H = H // 128  # 8 K-tiles to contract over a full hidden vector
NGATE = 4

_CACHE = {}


def _shard_host(inputs):
    """Build per-core input dicts (numpy only: slice / transpose / reshape)."""
    f32 = np.float32

    pre_x = np.asarray(inputs["pre_x"], f32)
    pre_y = np.asarray(inputs["pre_y"], f32)
    fwd_x = np.asarray(inputs["forward_x"], f32)

    # Encoder input, step-major, [t, p(128), kt(2), b] so the DMA is contiguous
    xy = np.concatenate([pre_x, pre_y], axis=2)  # (PRE, B, IN)
    xpre = (
        xy.transpose(0, 2, 1)  # (PRE, IN, B)
        .reshape(PRE_LEN, 2, 128, B)
        .transpose(0, 2, 1, 3)  # (PRE, 128, 2, B)
        .copy()
    )
    # Decoder exogenous input: [t, in(192), b]
    xfwd = fwd_x.transpose(0, 2, 1).copy()  # (FWD, 192, B)

    w_ih_0 = np.asarray(inputs["w_ih_0"], f32).reshape(NGATE, H, IN)
    w_hh_0 = np.asarray(inputs["w_hh_0"], f32).reshape(NGATE, H, H)
    w_ih_1 = np.asarray(inputs["w_ih_1"], f32).reshape(NGATE, H, H)
    w_hh_1 = np.asarray(inputs["w_hh_1"], f32).reshape(NGATE, H, H)
    b0 = (np.asarray(inputs["b_ih_0"], f32) + np.asarray(inputs["b_hh_0"], f32)).reshape(NGATE, H)
    b1 = (np.asarray(inputs["b_ih_1"], f32) + np.asarray(inputs["b_hh_1"], f32)).reshape(NGATE, H)
    fc_w1 = np.asarray(inputs["fc_w1"], f32)
    fc_b1 = np.asarray(inputs["fc_b1"], f32)
    fc_w2 = np.asarray(inputs["fc_w2"], f32)
    fc_b2 = np.asarray(inputs["fc_b2"], f32)

    def lhsT_hid(w, k):
        """(4, H, K) gate-major weight -> lhsT [128, KT, 4, 128] for core k."""
        sl = w[:, k * SL : (k + 1) * SL, :]  # (4, 128, K)
        kdim = sl.shape[2]
        kt = kdim // 128
        return (
            sl.transpose(2, 0, 1)  # (K, 4, 128)
            .reshape(kt, 128, NGATE, SL)
            .transpose(1, 0, 2, 3)  # (128, kt, 4, 128)
            .reshape(128, kt * NGATE * SL)
            .copy()
        )

    maps = []
    for k in range(NCORES):
        sl = slice(k * SL, (k + 1) * SL)
        w0xT = lhsT_hid(w_ih_0, k)  # (128, 2*4*128)
        west = w_ih_0[:, sl, XD:]  # (4, 128, 64)
        westT = west.transpose(2, 0, 1).reshape(YD, NGATE * SL).copy()  # (64, 512)
        whh0T = lhsT_hid(w_hh_0, k)  # (128, 8*4*128)
        wih1T = lhsT_hid(w_ih_1, k)
        whh1T = lhsT_hid(w_hh_1, k)
        # FC head is replicated on every core (small): removes the partials
        # AllGather from the decode critical path.
        fcw1T = (
            fc_w1.T.reshape(KT_H, 128, H).transpose(1, 0, 2).reshape(128, KT_H * H).copy()
        )
        fcw2T = (
            fc_w2.T.reshape(KT_H, 128, YD).transpose(1, 0, 2).reshape(128, KT_H * YD).copy()
        )
        m = {
            "xpre": xpre,
            "xfwd": xfwd,
            "w0xT": w0xT,
            "westT": westT,
            "whh0T": whh0T,
            "wih1T": wih1T,
            "whh1T": whh1T,
            "fcw1T": fcw1T,
            "fcw2T": fcw2T,
            "b0": b0[:, sl].T.copy(),  # (128, 4)
            "b1": b1[:, sl].T.copy(),
            "fcb1": fc_b1.reshape(KT_H, 128).T.copy(),  # (128, 8): bias per M-tile
            "fcb2": fc_b2.reshape(YD, 1).copy(),
            "lastyT": pre_y[-1].T.copy(),  # (64, 256)
        }
        maps.append(m)
    return maps


def _build_program():
    import concourse.bass as bass
    import concourse.mybir as mybir
    import concourse.tile as tile
    from concourse import bacc

    dt = mybir.dt
    AF = mybir.ActivationFunctionType
    F32 = dt.float32
    FR = dt.float32r  # rounded fp32: matmuls run at 1 cycle/row for N>=256

    nc = bacc.Bacc("TRN2", target_bir_lowering=False, debug=False, num_devices=NCORES)

    # ---- external I/O (everything feeding a matmul is declared float32r) ----
    t_xpre = nc.dram_tensor("xpre", [PRE_LEN, 128, 2, B], FR, kind="ExternalInput")
    t_xfwd = nc.dram_tensor("xfwd", [FWD_LEN, XD, B], FR, kind="ExternalInput")
    t_w0xT = nc.dram_tensor("w0xT", [128, 2 * NGATE * SL], FR, kind="ExternalInput")
    t_westT = nc.dram_tensor("westT", [YD, NGATE * SL], FR, kind="ExternalInput")
    t_whh0T = nc.dram_tensor("whh0T", [128, KT_H * NGATE * SL], FR, kind="ExternalInput")
    t_wih1T = nc.dram_tensor("wih1T", [128, KT_H * NGATE * SL], FR, kind="ExternalInput")
    t_whh1T = nc.dram_tensor("whh1T", [128, KT_H * NGATE * SL], FR, kind="ExternalInput")
    t_fcw1T = nc.dram_tensor("fcw1T", [128, KT_H * H], FR, kind="ExternalInput")
    t_fcw2T = nc.dram_tensor("fcw2T", [128, KT_H * YD], FR, kind="ExternalInput")
    t_b0 = nc.dram_tensor("b0", [128, NGATE], F32, kind="ExternalInput")
    t_b1 = nc.dram_tensor("b1", [128, NGATE], F32, kind="ExternalInput")
    t_fcb1 = nc.dram_tensor("fcb1", [128, KT_H], F32, kind="ExternalInput")
    t_fcb2 = nc.dram_tensor("fcb2", [YD, 1], F32, kind="ExternalInput")
    t_lastyT = nc.dram_tensor("lastyT", [YD, B], F32, kind="ExternalInput")
    t_out = nc.dram_tensor("est_out", [FWD_LEN, YD, B], F32, kind="ExternalOutput")

    RG = [list(range(NCORES))]

    with tile.TileContext(nc) as tc:
        with (
            tc.tile_pool(name="const", bufs=1) as const,
            tc.tile_pool(name="xload", bufs=3) as xload,
            tc.tile_pool(name="state", bufs=2) as state,
            tc.tile_pool(name="gact", bufs=3) as gact,
            tc.tile_pool(name="hfull", bufs=2) as hfull,
            tc.tile_pool(name="psum", bufs=8, space="PSUM") as psum,
            tc.tile_pool(name="dbounce", bufs=4, space="DRAM") as dbounce,
            tc.tile_pool(name="dshared", bufs=4, space="DRAM") as dshared,
        ):
            # ---- load constants ----
            w0xT = const.tile([128, 2, NGATE, SL], FR)
            nc.sync.dma_start(out=w0xT, in_=t_w0xT.ap().rearrange("p (k g m) -> p k g m", k=2, g=NGATE))
            westT = const.tile([YD, NGATE, SL], FR)
            nc.sync.dma_start(out=westT, in_=t_westT.ap().rearrange("p (g m) -> p g m", g=NGATE))
            whh0T = const.tile([128, KT_H, NGATE, SL], FR)
            nc.sync.dma_start(out=whh0T, in_=t_whh0T.ap().rearrange("p (k g m) -> p k g m", k=KT_H, g=NGATE))
            wih1T = const.tile([128, KT_H, NGATE, SL], FR)
            nc.sync.dma_start(out=wih1T, in_=t_wih1T.ap().rearrange("p (k g m) -> p k g m", k=KT_H, g=NGATE))
            whh1T = const.tile([128, KT_H, NGATE, SL], FR)
            nc.sync.dma_start(out=whh1T, in_=t_whh1T.ap().rearrange("p (k g m) -> p k g m", k=KT_H, g=NGATE))
            fcw1T = const.tile([128, KT_H, H], FR)
            nc.sync.dma_start(out=fcw1T, in_=t_fcw1T.ap().rearrange("p (k m) -> p k m", k=KT_H))
            fcw2T = const.tile([128, KT_H, YD], FR)
            nc.sync.dma_start(out=fcw2T, in_=t_fcw2T.ap().rearrange("p (k m) -> p k m", k=KT_H))
            b0 = const.tile([128, NGATE], F32)
            nc.sync.dma_start(out=b0, in_=t_b0.ap())
            b1 = const.tile([128, NGATE], F32)
            nc.sync.dma_start(out=b1, in_=t_b1.ap())
            fcb1 = const.tile([128, KT_H], F32)
            nc.sync.dma_start(out=fcb1, in_=t_fcb1.ap())
            fcb2 = const.tile([YD, 1], F32)
            nc.sync.dma_start(out=fcb2, in_=t_fcb2.ap())

            # ---- persistent state ----
            est = const.tile([YD, B], F32)  # replicated running estimate
            nc.sync.dma_start(out=est, in_=t_lastyT.ap())
            c0 = const.tile([128, B], F32)
            nc.vector.memset(c0, 0.0)
            c1 = const.tile([128, B], F32)
            nc.vector.memset(c1, 0.0)
            h0f = None  # gathered h0 of previous step [128, KT_H, B] (f32r)
            h1f = None  # gathered h1 of previous step

            def allgather(slice_ap, nparts, dtype):
                inb = dbounce.tile([nparts, B], dtype, tag="agin", name="agin")
                nc.sync.dma_start(out=inb, in_=slice_ap)
                outb = dshared.tile([NCORES * nparts, B], dtype, tag="agout",
                                    name="agout", addr_space="Shared")
                nc.gpsimd.collective_compute(
                    "AllGather",
                    mybir.AluOpType.bypass,
                    replica_groups=RG,
                    ins=[inb[:].opt()],
                    outs=[outb[:].opt()],
                )
                return outb

            def gather_to_sbuf(outb, nparts, dtype):
                """DMA gathered DRAM [NCORES*nparts, B] into SBUF [nparts, NCORES, B]
                k-tile by k-tile so consumers can start early."""
                full = hfull.tile([nparts, NCORES, B], dtype, tag=f"full{nparts}",
                                  name="hfull")
                src = outb[:].rearrange("(k p) b -> p k b", p=nparts)
                for k in range(NCORES):
                    nc.sync.dma_start(out=full[:, k, :], in_=src[:, k, :])
                return full

            def lstm_halfstep(zp, bias, cprev, tagp):
                """Gate activations + cell update. zp: 4 PSUM tiles [128,B].
                Returns (c_new, h_new); h_new is written as float32r."""
                gi = gact.tile([128, B], F32, tag="gi", name="gi")
                gf = gact.tile([128, B], F32, tag="gf", name="gf")
                gg = gact.tile([128, B], F32, tag="gg", name="gg")
                go = gact.tile([128, B], F32, tag="go", name="go")
                nc.scalar.activation(gi, zp[0], AF.Sigmoid, bias=bias[:, 0:1])
                nc.scalar.activation(gf, zp[1], AF.Sigmoid, bias=bias[:, 1:2])
                nc.scalar.activation(gg, zp[2], AF.Tanh, bias=bias[:, 2:3])
                nc.scalar.activation(go, zp[3], AF.Sigmoid, bias=bias[:, 3:4])
                fc_ = gact.tile([128, B], F32, tag="fc_", name="fc_")
                nc.vector.tensor_mul(fc_, gf, cprev)
                ig = gact.tile([128, B], F32, tag="ig", name="ig")
                nc.vector.tensor_mul(ig, gi, gg)
                cnew = state.tile([128, B], F32, tag=tagp, name="cnew")
                nc.vector.tensor_add(cnew, fc_, ig)
                tc_ = gact.tile([128, B], F32, tag="tc_", name="tc_")
                nc.scalar.activation(tc_, cnew, AF.Tanh)
                hnew = state.tile([128, B], FR, tag=tagp + "h", name="hnew")
                nc.vector.tensor_mul(hnew, go, tc_)
                return cnew, hnew

            nsteps = PRE_LEN + FWD_LEN
            for s in range(nsteps):
                dec = s >= PRE_LEN
                # ---------- layer0 gate matmuls ----------
                z0 = [psum.tile([128, B], F32, tag="z", name=f"z0g{g}") for g in range(NGATE)]
                xt = xload.tile([128, 2, B], FR, tag="x", name="xt")
                if not dec:
                    nc.sync.dma_start(out=xt, in_=t_xpre.ap()[s])
                    for g in range(NGATE):
                        nc.tensor.matmul(z0[g], w0xT[:, 0, g, :], xt[:, 0, :],
                                         start=True, stop=False)
                        nc.tensor.matmul(z0[g], w0xT[:, 1, g, :], xt[:, 1, :],
                                         start=False, stop=(s == 0))
                        if s > 0:
                            for k in range(KT_H):
                                nc.tensor.matmul(
                                    z0[g], whh0T[:, k, g, :], h0f[:, k, :],
                                    start=False, stop=(k == KT_H - 1),
                                )
                else:
                    sf = s - PRE_LEN
                    nc.sync.dma_start(out=xt[:, 0, :], in_=t_xfwd.ap()[sf, 0:128, :])
                    nc.sync.dma_start(out=xt[0:64, 1, :], in_=t_xfwd.ap()[sf, 128:XD, :])
                    # FC head (replicated on every core, no collective):
                    # est_s = fc(h1_{s-1}) + est_{s-1}; h1f holds the gathered
                    # previous h1.  u computed M-tile by M-tile, then the
                    # second matmul contracts over u's M-tiles as K-tiles.
                    u = gact.tile([128, KT_H, B], FR, tag="u", name="u")
                    for m in range(KT_H):
                        up = psum.tile([128, B], F32, tag="z", name="up")
                        for k in range(KT_H):
                            nc.tensor.matmul(up, fcw1T[:, k, m * 128:(m + 1) * 128],
                                             h1f[:, k, :],
                                             start=(k == 0), stop=(k == KT_H - 1))
                        nc.scalar.activation(u[:, m, :], up, AF.Tanh,
                                             bias=fcb1[:, m:m + 1])
                    pp = psum.tile([YD, B], F32, tag="z", name="pp")
                    for k in range(KT_H):
                        nc.tensor.matmul(pp, fcw2T[:, k, :], u[:, k, :],
                                         start=(k == 0), stop=(k == KT_H - 1))
                    estn = state.tile([YD, B], F32, tag="est", name="estn")
                    nc.vector.tensor_add(estn, est, pp)
                    nc.vector.tensor_scalar_add(estn, estn, fcb2[:, 0:1])
                    est = estn
                    est_r = state.tile([YD, B], FR, tag="estr", name="est_r")
                    nc.vector.tensor_copy(est_r, est)
                    nc.sync.dma_start(out=t_out.ap()[sf], in_=est)
                    for g in range(NGATE):
                        nc.tensor.matmul(z0[g], w0xT[:, 0, g, :], xt[:, 0, :],
                                         start=True, stop=False)
                        nc.tensor.matmul(z0[g], w0xT[0:64, 1, g, :], xt[0:64, 1, :],
                                         start=False, stop=False)
                        for k in range(KT_H):
                            nc.tensor.matmul(z0[g], whh0T[:, k, g, :], h0f[:, k, :],
                                             start=False, stop=False)
                        nc.tensor.matmul(z0[g], westT[:, g, :], est_r,
                                         start=False, stop=True)

                c0, h0k = lstm_halfstep(z0, b0, c0, "c0")
                g0 = allgather(h0k[:], 128, FR)
                h0f = gather_to_sbuf(g0, 128, FR)

                # ---------- layer1 ----------
                z1 = [psum.tile([128, B], F32, tag="z", name=f"z1g{g}") for g in range(NGATE)]
                for g in range(NGATE):
                    if s > 0:
                        for k in range(KT_H):
                            nc.tensor.matmul(z1[g], whh1T[:, k, g, :], h1f[:, k, :],
                                             start=(k == 0), stop=False)
                    for k in range(KT_H):
                        nc.tensor.matmul(z1[g], wih1T[:, k, g, :], h0f[:, k, :],
                                         start=(s == 0 and k == 0), stop=(k == KT_H - 1))
                c1, h1k = lstm_halfstep(z1, b1, c1, "c1")
                if s < nsteps - 1:
                    g1 = allgather(h1k[:], 128, FR)
                    h1f = gather_to_sbuf(g1, 128, FR)

    nc.compile()
    return nc


def kernel(**inputs) -> np.ndarray:
    from concourse.bass_utils import run_bass_kernel_spmd

    key = "prog"
    if key not in _CACHE:
        _CACHE[key] = _build_program()
    nc = _CACHE[key]

    in_maps = _shard_host(inputs)
    res = run_bass_kernel_spmd(nc, in_maps, core_ids=list(range(NCORES)))
    est = np.asarray(res.results[0]["est_out"])  # (FWD, YD, B)
    return est.transpose(0, 2, 1).astype(np.float32).copy()  # (FWD, B, YD)


# revision 23
# speedup vs baseline: 277.8954x; 1.4856x over previous
"""Trainium2 Bass kernel for nn_DiffNet (2-layer LSTM encoder/decoder + FC head).

Sharding: tensor-parallel over the hidden/gate dimension across 8 NeuronCores.
Core k owns hidden rows [k*128, (k+1)*128) of both LSTM layers (and the
matching rows of each of the 4 gates) plus the matching rows of fc_w1 and
columns of fc_w2.  Activations are stored as [hidden_partitions, batch] so the
full batch (256) is the matmul moving dimension; float32r matmuls then run at
1 cycle/row.  Hidden states are exchanged once per layer per step with an
8-rank AllGather; the decode FC contribution is computed K-sharded and its
partial products ride a third AllGather, summed on every core so `est` is
replicated.

Self-contained: hardcodes all shapes; host-side numpy only reshapes/slices.
"""

import os

import numpy as np

L = 2
H = 1024
XD = 192
YD = 64
IN = XD + YD  # 256
B = 256
PRE_LEN = int(os.environ.get("DIFFNET_PRE", "64"))
FWD_LEN = int(os.environ.get("DIFFNET_FWD", "48"))
NCORES = 8
SL = H // NCORES  # 128 hidden rows per core
KT_H = H // 128  # 8 K-tiles to contract over a full hidden vector
NGATE = 4
# ablation knob (timing experiments only — results are numerically wrong):
# 1 = skip collective_compute, read local bounce instead (same DMA geometry)
_NOCC = os.environ.get("DIFFNET_NOCC", "0") == "1"

_CACHE = {}


def _shard_host(inputs):
    """Build per-core input dicts (numpy only: slice / transpose / reshape)."""
    f32 = np.float32

    pre_x = np.asarray(inputs["pre_x"], f32)
    pre_y = np.asarray(inputs["pre_y"], f32)
    fwd_x = np.asarray(inputs["forward_x"], f32)

    # Encoder input, step-major, [t, p(128), kt(2), b] so the DMA is contiguous
    xy = np.concatenate([pre_x, pre_y], axis=2)  # (PRE, B, IN)
    xpre = (
        xy.transpose(0, 2, 1)  # (PRE, IN, B)
        .reshape(PRE_LEN, 2, 128, B)
        .transpose(0, 2, 1, 3)  # (PRE, 128, 2, B)
        .astype(np.float16)
    )
    # Decoder exogenous input: [t, in(192), b]
    xfwd = fwd_x.transpose(0, 2, 1).astype(np.float16)  # (FWD, 192, B)

    w_ih_0 = np.asarray(inputs["w_ih_0"], f32).reshape(NGATE, H, IN)
    w_hh_0 = np.asarray(inputs["w_hh_0"], f32).reshape(NGATE, H, H)
    w_ih_1 = np.asarray(inputs["w_ih_1"], f32).reshape(NGATE, H, H)
    w_hh_1 = np.asarray(inputs["w_hh_1"], f32).reshape(NGATE, H, H)
    b0 = (np.asarray(inputs["b_ih_0"], f32) + np.asarray(inputs["b_hh_0"], f32)).reshape(NGATE, H)
    b1 = (np.asarray(inputs["b_ih_1"], f32) + np.asarray(inputs["b_hh_1"], f32)).reshape(NGATE, H)
    fc_w1 = np.asarray(inputs["fc_w1"], f32)
    fc_b1 = np.asarray(inputs["fc_b1"], f32)
    fc_w2 = np.asarray(inputs["fc_w2"], f32)
    fc_b2 = np.asarray(inputs["fc_b2"], f32)

    def lhsT_hid(w, k):
        """(4, H, K) gate-major weight -> lhsT [128, KT, 4, 128] for core k."""
        sl = w[:, k * SL : (k + 1) * SL, :]  # (4, 128, K)
        kdim = sl.shape[2]
        kt = kdim // 128
        return (
            sl.transpose(2, 0, 1)  # (K, 4, 128)
            .reshape(kt, 128, NGATE, SL)
            .transpose(1, 0, 2, 3)  # (128, kt, 4, 128)
            .reshape(128, kt * NGATE * SL)
            .copy()
        )

    maps = []
    for k in range(NCORES):
        sl = slice(k * SL, (k + 1) * SL)
        w0xT = lhsT_hid(w_ih_0, k)  # (128, 2*4*128)
        west = w_ih_0[:, sl, XD:]  # (4, 128, 64)
        westT = west.transpose(2, 0, 1).reshape(YD, NGATE * SL).copy()  # (64, 512)
        whh0T = lhsT_hid(w_hh_0, k)  # (128, 8*4*128)
        wih1T = lhsT_hid(w_ih_1, k)
        whh1T = lhsT_hid(w_hh_1, k)
        # FC head is replicated on every core (small): removes the partials
        # AllGather from the decode critical path.
        fcw1T = (
            fc_w1.T.reshape(KT_H, 128, H).transpose(1, 0, 2).reshape(128, KT_H * H).copy()
        )
        fcw2T = (
            fc_w2.T.reshape(KT_H, 128, YD).transpose(1, 0, 2).reshape(128, KT_H * YD).copy()
        )
        m = {
            "xpre": xpre,
            "xfwd": xfwd,
            "w0xT": w0xT.astype(np.float16),
            "westT": westT.astype(np.float16),
            "whh0T": whh0T.astype(np.float16),
            "wih1T": wih1T.astype(np.float16),
            "whh1T": whh1T.astype(np.float16),
            "fcw1T": fcw1T.astype(np.float16),
            "fcw2T": fcw2T.astype(np.float16),
            "b0": b0[:, sl].T.copy(),  # (128, 4)
            "b1": b1[:, sl].T.copy(),
            "fcb1": fc_b1.reshape(KT_H, 128).T.copy(),  # (128, 8): bias per M-tile
            "fcb2": fc_b2.reshape(YD, 1).copy(),
            "lastyT": pre_y[-1].T.copy(),  # (64, 256)
        }
        maps.append(m)
    return maps


def _build_program():
    import concourse.bass as bass
    import concourse.mybir as mybir
    import concourse.tile as tile
    from concourse import bacc

    dt = mybir.dt
    AF = mybir.ActivationFunctionType
    F32 = dt.float32
    FR = dt.float16  # matmul operand dtype.
    # fp32/f32r stationary weights stall the PE ~4.7us per matmul (FWL is
    # disabled for 4-byte dtypes and every matmul here loads fresh weights);
    # bf16 avoids the stall but its 7-bit mantissa compounds to 5.2e-2 error
    # over 112 recurrent steps. float16 (10-bit mantissa, all values in range
    # here) keeps FWL on at ~8x the precision of bf16.

    nc = bacc.Bacc("TRN2", target_bir_lowering=False, debug=False, num_devices=NCORES)

    # ---- external I/O (everything feeding a matmul is declared float32r) ----
    t_xpre = nc.dram_tensor("xpre", [PRE_LEN, 128, 2, B], FR, kind="ExternalInput")
    t_xfwd = nc.dram_tensor("xfwd", [FWD_LEN, XD, B], FR, kind="ExternalInput")
    t_w0xT = nc.dram_tensor("w0xT", [128, 2 * NGATE * SL], FR, kind="ExternalInput")
    t_westT = nc.dram_tensor("westT", [YD, NGATE * SL], FR, kind="ExternalInput")
    t_whh0T = nc.dram_tensor("whh0T", [128, KT_H * NGATE * SL], FR, kind="ExternalInput")
    t_wih1T = nc.dram_tensor("wih1T", [128, KT_H * NGATE * SL], FR, kind="ExternalInput")
    t_whh1T = nc.dram_tensor("whh1T", [128, KT_H * NGATE * SL], FR, kind="ExternalInput")
    t_fcw1T = nc.dram_tensor("fcw1T", [128, KT_H * H], FR, kind="ExternalInput")
    t_fcw2T = nc.dram_tensor("fcw2T", [128, KT_H * YD], FR, kind="ExternalInput")
    t_b0 = nc.dram_tensor("b0", [128, NGATE], F32, kind="ExternalInput")
    t_b1 = nc.dram_tensor("b1", [128, NGATE], F32, kind="ExternalInput")
    t_fcb1 = nc.dram_tensor("fcb1", [128, KT_H], F32, kind="ExternalInput")
    t_fcb2 = nc.dram_tensor("fcb2", [YD, 1], F32, kind="ExternalInput")
    t_lastyT = nc.dram_tensor("lastyT", [YD, B], F32, kind="ExternalInput")
    t_out = nc.dram_tensor("est_out", [FWD_LEN, YD, B], F32, kind="ExternalOutput")

    RG = [list(range(NCORES))]

    with tile.TileContext(nc) as tc:
        with (
            tc.tile_pool(name="const", bufs=1) as const,
            tc.tile_pool(name="xload", bufs=3) as xload,
            tc.tile_pool(name="state", bufs=2) as state,
            tc.tile_pool(name="gact", bufs=3) as gact,
            tc.tile_pool(name="hfull", bufs=2) as hfull,
            tc.tile_pool(name="psum", bufs=8, space="PSUM") as psum,
            tc.tile_pool(name="dbounce", bufs=4, space="DRAM") as dbounce,
            tc.tile_pool(name="dshared", bufs=4, space="DRAM") as dshared,
        ):
            # ---- load constants ----
            w0xT = const.tile([128, 2, NGATE, SL], FR)
            nc.sync.dma_start(out=w0xT, in_=t_w0xT.ap().rearrange("p (k g m) -> p k g m", k=2, g=NGATE))
            westT = const.tile([YD, NGATE, SL], FR)
            nc.sync.dma_start(out=westT, in_=t_westT.ap().rearrange("p (g m) -> p g m", g=NGATE))
            whh0T = const.tile([128, KT_H, NGATE, SL], FR)
            nc.sync.dma_start(out=whh0T, in_=t_whh0T.ap().rearrange("p (k g m) -> p k g m", k=KT_H, g=NGATE))
            wih1T = const.tile([128, KT_H, NGATE, SL], FR)
            nc.sync.dma_start(out=wih1T, in_=t_wih1T.ap().rearrange("p (k g m) -> p k g m", k=KT_H, g=NGATE))
            whh1T = const.tile([128, KT_H, NGATE, SL], FR)
            nc.sync.dma_start(out=whh1T, in_=t_whh1T.ap().rearrange("p (k g m) -> p k g m", k=KT_H, g=NGATE))
            fcw1T = const.tile([128, KT_H, H], FR)
            nc.sync.dma_start(out=fcw1T, in_=t_fcw1T.ap().rearrange("p (k m) -> p k m", k=KT_H))
            fcw2T = const.tile([128, KT_H, YD], FR)
            nc.sync.dma_start(out=fcw2T, in_=t_fcw2T.ap().rearrange("p (k m) -> p k m", k=KT_H))
            b0 = const.tile([128, NGATE], F32)
            nc.sync.dma_start(out=b0, in_=t_b0.ap())
            b1 = const.tile([128, NGATE], F32)
            nc.sync.dma_start(out=b1, in_=t_b1.ap())
            fcb1 = const.tile([128, KT_H], F32)
            nc.sync.dma_start(out=fcb1, in_=t_fcb1.ap())
            fcb2 = const.tile([YD, 1], F32)
            nc.sync.dma_start(out=fcb2, in_=t_fcb2.ap())

            # ---- persistent state ----
            est = const.tile([YD, B], F32)  # replicated running estimate
            nc.sync.dma_start(out=est, in_=t_lastyT.ap())
            c0 = const.tile([128, B], F32)
            nc.vector.memset(c0, 0.0)
            c1 = const.tile([128, B], F32)
            nc.vector.memset(c1, 0.0)
            h0f = None  # gathered h0 of previous step [128, KT_H, B] (f32r)
            h1f = None  # gathered h1 of previous step

            def allgather(slice_ap, nparts, dtype):
                inb = dbounce.tile([nparts, B], dtype, tag="agin", name="agin")
                nc.sync.dma_start(out=inb, in_=slice_ap)
                if _NOCC:
                    return inb
                outb = dshared.tile([NCORES * nparts, B], dtype, tag="agout",
                                    name="agout", addr_space="Shared")
                nc.gpsimd.collective_compute(
                    "AllGather",
                    mybir.AluOpType.bypass,
                    replica_groups=RG,
                    ins=[inb[:].opt()],
                    outs=[outb[:].opt()],
                )
                return outb

            def gather_to_sbuf(outb, nparts, dtype):
                """DMA gathered DRAM [NCORES*nparts, B] into SBUF [nparts, NCORES, B]
                k-tile by k-tile so consumers can start early."""
                full = hfull.tile([nparts, NCORES, B], dtype, tag=f"full{nparts}",
                                  name="hfull")
                if _NOCC:
                    import concourse.bass as bass_mod
                    src = outb[:]
                    src = bass_mod.AP(tensor=src.tensor, offset=src.offset,
                                      ap=[src.ap[0], [0, NCORES]] + src.ap[1:])
                    nc.sync.dma_start(out=full[:, :, :], in_=src)
                    return full
                src = outb[:].rearrange("(k p) b -> p k b", p=nparts)
                nc.sync.dma_start(out=full[:, :, :], in_=src)
                return full

            def lstm_halfstep(zp, bias, cprev, tagp):
                """Gate activations + cell update. zp: 4 PSUM tiles [128,B].
                Returns (c_new, h_new); h_new is written as float32r."""
                # sigmoids grouped together, tanh last: fewer ScalarE
                # activation-table swaps per step
                gi = gact.tile([128, B], F32, tag="gi", name="gi")
                gf = gact.tile([128, B], F32, tag="gf", name="gf")
                gg = gact.tile([128, B], F32, tag="gg", name="gg")
                go = gact.tile([128, B], F32, tag="go", name="go")
                nc.scalar.activation(gi, zp[0], AF.Sigmoid, bias=bias[:, 0:1])
                nc.scalar.activation(gf, zp[1], AF.Sigmoid, bias=bias[:, 1:2])
                nc.scalar.activation(go, zp[3], AF.Sigmoid, bias=bias[:, 3:4])
                nc.scalar.activation(gg, zp[2], AF.Tanh, bias=bias[:, 2:3])
                fc_ = gact.tile([128, B], F32, tag="fc_", name="fc_")
                nc.vector.tensor_mul(fc_, gf, cprev)
                ig = gact.tile([128, B], F32, tag="ig", name="ig")
                nc.vector.tensor_mul(ig, gi, gg)
                cnew = state.tile([128, B], F32, tag=tagp, name="cnew")
                nc.vector.tensor_add(cnew, fc_, ig)
                tc_ = gact.tile([128, B], F32, tag="tc_", name="tc_")
                nc.scalar.activation(tc_, cnew, AF.Tanh)
                hnew = state.tile([128, B], FR, tag=tagp + "h", name="hnew")
                nc.vector.tensor_mul(hnew, go, tc_)
                return cnew, hnew

            # ================= encode: skewed bodies, ONE AllGather per step
            # body s computes L1 of step s-1 (from h0f=h0_{s-1}, h1f=h1_{s-2})
            # and L0 of step s, then gathers [h0_s ; h1_{s-1}] in one
            # collective.
            h1k = None
            for s in range(PRE_LEN):
                # ---- L1_{s-1} ----
                if s >= 1:
                    z1 = [psum.tile([128, B], F32, tag="z", name=f"z1g{g}") for g in range(NGATE)]
                    for g in range(NGATE):
                        if s >= 2:
                            for k in range(KT_H):
                                nc.tensor.matmul(z1[g], whh1T[:, k, g, :], h1f[:, k, :],
                                                 start=(k == 0), stop=False)
                        for k in range(KT_H):
                            nc.tensor.matmul(z1[g], wih1T[:, k, g, :], h0f[:, k, :],
                                             start=(s == 1 and k == 0), stop=(k == KT_H - 1))
                    c1, h1k = lstm_halfstep(z1, b1, c1, "c1")
                # ---- L0_s ----
                z0 = [psum.tile([128, B], F32, tag="z", name=f"z0g{g}") for g in range(NGATE)]
                xt = xload.tile([128, 2, B], FR, tag="x", name="xt")
                nc.sync.dma_start(out=xt, in_=t_xpre.ap()[s])
                for g in range(NGATE):
                    nc.tensor.matmul(z0[g], w0xT[:, 0, g, :], xt[:, 0, :],
                                     start=True, stop=False)
                    nc.tensor.matmul(z0[g], w0xT[:, 1, g, :], xt[:, 1, :],
                                     start=False, stop=(s == 0))
                    if s > 0:
                        for k in range(KT_H):
                            nc.tensor.matmul(z0[g], whh0T[:, k, g, :], h0f[:, k, :],
                                             start=False, stop=(k == KT_H - 1))
                c0, h0k = lstm_halfstep(z0, b0, c0, "c0")

                # ---- merged AllGather [h0_s ; h1_{s-1}] ----
                inb = dbounce.tile([2 * 128, B], FR, tag="agin", name="agin")
                nc.sync.dma_start(out=inb[0:128, :], in_=h0k[:])
                nc.sync.dma_start(out=inb[128:256, :], in_=(h1k if s >= 1 else h0k)[:])
                if _NOCC:
                    src = inb[:].rearrange("(h p) b -> p h b", h=2)
                    src = bass.AP(tensor=src.tensor, offset=src.offset,
                                  ap=[src.ap[0], [0, NCORES]] + src.ap[1:])
                else:
                    outb = dshared.tile([NCORES * 2 * 128, B], FR, tag="agout",
                                        name="agout", addr_space="Shared")
                    nc.gpsimd.collective_compute(
                        "AllGather", mybir.AluOpType.bypass, replica_groups=RG,
                        ins=[inb[:].opt()], outs=[outb[:].opt()],
                    )
                    src = outb[:].rearrange("(k h p) b -> p k h b", k=NCORES, h=2)
                h0f = hfull.tile([128, NCORES, B], FR, tag="h0f", name="h0f")
                nc.sync.dma_start(out=h0f[:, :, :], in_=src[:, :, 0, :])
                if s >= 1:
                    h1f = hfull.tile([128, NCORES, B], FR, tag="h1f", name="h1f")
                    nc.sync.dma_start(out=h1f[:, :, :], in_=src[:, :, 1, :])

            # ---- encode flush: L1 of the last encode step (h1_63 = "top") ----
            z1 = [psum.tile([128, B], F32, tag="z", name=f"zfg{g}") for g in range(NGATE)]
            for g in range(NGATE):
                for k in range(KT_H):
                    nc.tensor.matmul(z1[g], whh1T[:, k, g, :], h1f[:, k, :],
                                     start=(k == 0), stop=False)
                for k in range(KT_H):
                    nc.tensor.matmul(z1[g], wih1T[:, k, g, :], h0f[:, k, :],
                                     start=False, stop=(k == KT_H - 1))
            c1, h1k = lstm_halfstep(z1, b1, c1, "c1")
            g1 = allgather(h1k[:], 128, FR)
            h1f = gather_to_sbuf(g1, 128, FR)

            # ================= decode =================
            for t in range(FWD_LEN):
                last = t == FWD_LEN - 1
                # ---- L0 part that doesn't need est: emitted first so the PE
                # works while the h1 gather (needed by the FC) is in flight.
                if not last:
                    z0 = [psum.tile([128, B], F32, tag="z", name=f"z0g{g}") for g in range(NGATE)]
                    xt = xload.tile([128, 2, B], FR, tag="x", name="xt")
                    nc.sync.dma_start(out=xt[:, 0, :], in_=t_xfwd.ap()[t, 0:128, :])
                    nc.sync.dma_start(out=xt[0:64, 1, :], in_=t_xfwd.ap()[t, 128:XD, :])
                    for g in range(NGATE):
                        nc.tensor.matmul(z0[g], w0xT[:, 0, g, :], xt[:, 0, :],
                                         start=True, stop=False)
                        nc.tensor.matmul(z0[g], w0xT[0:64, 1, g, :], xt[0:64, 1, :],
                                         start=False, stop=False)
                        for k in range(KT_H):
                            nc.tensor.matmul(z0[g], whh0T[:, k, g, :], h0f[:, k, :],
                                             start=False, stop=False)

                # ---- FC head (replicated, no collective): est_t ----
                u = gact.tile([128, KT_H, B], FR, tag="u", name="u")
                for m in range(KT_H):
                    up = psum.tile([128, B], F32, tag="z", name="up")
                    for k in range(KT_H):
                        nc.tensor.matmul(up, fcw1T[:, k, m * 128:(m + 1) * 128],
                                         h1f[:, k, :],
                                         start=(k == 0), stop=(k == KT_H - 1))
                    nc.scalar.activation(u[:, m, :], up, AF.Tanh,
                                         bias=fcb1[:, m:m + 1])
                pp = psum.tile([YD, B], F32, tag="z", name="pp")
                for k in range(KT_H):
                    nc.tensor.matmul(pp, fcw2T[:, k, :], u[:, k, :],
                                     start=(k == 0), stop=(k == KT_H - 1))
                estn = state.tile([YD, B], F32, tag="est", name="estn")
                nc.vector.tensor_add(estn, est, pp)
                nc.vector.tensor_scalar_add(estn, estn, fcb2[:, 0:1])
                est = estn
                nc.sync.dma_start(out=t_out.ap()[t], in_=est)
                if last:
                    break
                est_r = state.tile([YD, B], FR, tag="estr", name="est_r")
                nc.vector.tensor_copy(est_r, est)
                # close the z0 accumulation with the est K-tile
                for g in range(NGATE):
                    nc.tensor.matmul(z0[g], westT[:, g, :], est_r,
                                     start=False, stop=True)
                c0, h0k = lstm_halfstep(z0, b0, c0, "c0")
                g0 = allgather(h0k[:], 128, FR)
                h0f = gather_to_sbuf(g0, 128, FR)

                # ---- L1 ----
                z1 = [psum.tile([128, B], F32, tag="z", name=f"z1g{g}") for g in range(NGATE)]
                for g in range(NGATE):
                    for k in range(KT_H):
                        nc.tensor.matmul(z1[g], whh1T[:, k, g, :], h1f[:, k, :],
                                         start=(k == 0), stop=False)
                    for k in range(KT_H):
                        nc.tensor.matmul(z1[g], wih1T[:, k, g, :], h0f[:, k, :],
                                         start=False, stop=(k == KT_H - 1))
                c1, h1k = lstm_halfstep(z1, b1, c1, "c1")
                g1 = allgather(h1k[:], 128, FR)
                h1f = gather_to_sbuf(g1, 128, FR)

    nc.compile()
    return nc


def kernel(**inputs) -> np.ndarray:
    from concourse.bass_utils import run_bass_kernel_spmd

    key = "prog"
    if key not in _CACHE:
        _CACHE[key] = _build_program()
    nc = _CACHE[key]

    in_maps = _shard_host(inputs)
    res = run_bass_kernel_spmd(nc, in_maps, core_ids=list(range(NCORES)))
    est = np.asarray(res.results[0]["est_out"])  # (FWD, YD, B)
    return est.transpose(0, 2, 1).astype(np.float32).copy()  # (FWD, B, YD)
